# revision 9
# baseline (speedup 1.0000x reference)
"""AttSeqDecoder Trainium2 kernel: GRU decoder with attention + tied-embedding logits.

Sharding: data-parallel over batch (B=32 -> 8 cores x 4). Each core runs the
full 64-step recurrence for its 4 batch rows, then projects its [256, 512]
outputs against the full [512, 32000] tied embedding for logits.
"""

import sys

sys.path.insert(0, "/opt/trn_rl_repo")

import numpy as np

import concourse.bass as bass
import concourse.mybir as mybir
from concourse import bacc
from concourse.tile import TileContext
from concourse.bass_utils import run_bass_kernel_spmd

F32 = mybir.dt.float32
F32R = mybir.dt.float32r
I32 = mybir.dt.int32

V, D, R, E, B, T, S = 32000, 512, 512, 512, 32, 64, 64
NC_ = 8
BL = B // NC_  # 4 batch rows per core
TB = T * BL  # 256 (t, b) rows per core

_BUILT = {}


def build_kernel(t_steps=T, v_out=V):
    nc = bacc.Bacc("TRN2", target_bir_lowering=False, debug=False, num_devices=NC_)
    tb = t_steps * BL

    # ---- DRAM parameters -------------------------------------------------
    def inp(name, shape, dt=F32):
        return nc.declare_dram_parameter(name, list(shape), dt, isOutput=False)

    emb_dram = inp("emb_W", (V, D))
    embT_dram = inp("embT", (D, v_out))
    idx_dram = inp("idx", (tb, 1), I32)
    wihe_dram = inp("WiheT", (D, 3 * R))
    wiho_dram = inp("WihoT", (D, 3 * R))
    whh_dram = inp("WhhT", (R, 3 * R))
    wh_dram = inp("WhT", (R, D))
    wc_dram = inp("WcT", (E, D))
    wa_dram = inp("WaT", (E, R))
    encr_dram = inp("enc_r", (BL * S, E))
    enct_dram = inp("encT", (E, BL * S))
    gxeb_dram = inp("gxe_bias", (1, 3 * R))
    ghnb_dram = inp("ghn_bias", (1, R))
    ob_dram = inp("o_bias", (1, D))
    blog_dram = inp("b_logits", (1, v_out))
    dmask_dram = inp("diagmask", (BL, BL * S))
    h0_dram = inp("h0", (BL, R))
    h0t_dram = inp("h0T", (128, 4 * BL))
    i4_dram = inp("I4", (BL, BL))
    i128_dram = inp("I128", (128, 128))
    ones_dram = inp("ones128", (1, 128))
    out_dram = nc.declare_dram_parameter("out", [tb, v_out], F32, isOutput=True)

    NT = (v_out + 511) // 512  # logits n-tiles
    MT = (tb + 127) // 128  # logits m-tiles

    with TileContext(nc) as tc:
        with tc.tile_pool(name="const", bufs=1) as cp, \
             tc.tile_pool(name="states", bufs=1) as sp:
            # ---- persistent loads (cast f32 -> f32r via gpsimd dma) ------
            wiho = cp.tile([128, 4 * 1536], F32R)
            whh = cp.tile([128, 4 * 1536], F32R)
            wht = cp.tile([128, 4 * 512], F32R)
            for j in range(4):
                nc.gpsimd.dma_start(wiho[:, 1536 * j : 1536 * (j + 1)], wiho_dram[128 * j : 128 * (j + 1), :])
                nc.gpsimd.dma_start(whh[:, 1536 * j : 1536 * (j + 1)], whh_dram[128 * j : 128 * (j + 1), :])
                nc.gpsimd.dma_start(wht[:, 512 * j : 512 * (j + 1)], wh_dram[128 * j : 128 * (j + 1), :])
            wct = cp.tile([128, 4 * 512], F32R)
            wat = cp.tile([128, 4 * 512], F32R)
            enct = cp.tile([128, 4 * 256], F32R)
            for j in range(4):
                nc.gpsimd.dma_start(wct[:, 512 * j : 512 * (j + 1)], wc_dram[128 * j : 128 * (j + 1), :])
                nc.gpsimd.dma_start(wat[:, 512 * j : 512 * (j + 1)], wa_dram[128 * j : 128 * (j + 1), :])
                nc.gpsimd.dma_start(enct[:, 256 * j : 256 * (j + 1)], enct_dram[128 * j : 128 * (j + 1), :])
            encr = cp.tile([128, 2 * 512], F32R)
            for i in range(2):
                nc.gpsimd.dma_start(encr[:, 512 * i : 512 * (i + 1)], encr_dram[128 * i : 128 * (i + 1), :])
            gxeb = cp.tile([1, 1536], F32R)
            ghnb = cp.tile([1, 512], F32R)
            obias = cp.tile([1, 512], F32R)
            nc.gpsimd.dma_start(gxeb[:, :], gxeb_dram[:, :])
            nc.gpsimd.dma_start(ghnb[:, :], ghnb_dram[:, :])
            nc.gpsimd.dma_start(obias[:, :], ob_dram[:, :])
            dmask = cp.tile([BL, BL * S], F32)
            nc.sync.dma_start(dmask[:, :], dmask_dram[:, :])
            i4 = cp.tile([BL, BL], F32R)
            nc.gpsimd.dma_start(i4[:, :], i4_dram[:, :])
            ident = cp.tile([128, 128], F32)
            nc.sync.dma_start(ident[:, :], i128_dram[:, :])
            ones = cp.tile([1, 128], F32R)
            nc.gpsimd.dma_start(ones[:, :], ones_dram[:, :])
            i128r = cp.tile([128, 128], F32R)
            nc.gpsimd.dma_start(i128r[:, :], i128_dram[:, :])

            h0t = sp.tile([128, 4 * BL], F32R)
            nc.gpsimd.dma_start(h0t[:, :], h0t_dram[:, :])
            h0sb = sp.tile([BL, R], F32)
            nc.sync.dma_start(h0sb[:, :], h0_dram[:, :])
            oT_all = sp.tile([128, t_steps * 4 * BL], F32R)

            # ---- setup compute ------------------------------------------
            n_half = (tb + 127) // 128
            with tc.tile_pool(name="setup", bufs=1) as stp, \
                 tc.tile_pool(name="setup_ps", bufs=2, space="PSUM") as spp:
                # embedding gather + transpose -> eT [128, (j, tb)]
                eT = stp.tile([128, 4 * tb], F32R)
                for half in range(n_half):
                    rows = min(128, tb - 128 * half)
                    idx_sb = stp.tile([128, 1], I32, tag="idx")
                    nc.sync.dma_start(idx_sb[:rows, :], idx_dram[128 * half : 128 * half + rows, :])
                    e_sb = stp.tile([128, D], F32, tag="esb")
                    nc.gpsimd.indirect_dma_start(
                        out=e_sb[:rows, :], out_offset=None, in_=emb_dram[:, :],
                        in_offset=bass.IndirectOffsetOnAxis(ap=idx_sb[:rows, :1], axis=0),
                    )
                    for j in range(4):
                        pt = spp.tile([128, 128], F32, tag="pt")
                        nc.tensor.transpose(pt[:, :rows], e_sb[:rows, 128 * j : 128 * (j + 1)], ident[:rows, :rows])
                        nc.vector.tensor_copy(eT[:, tb * j + 128 * half : tb * j + 128 * half + rows], pt[:, :rows])
                # gx_e = e @ WiheT + gxe_bias -> gxe [128, (m, f)]
                gxe = sp.tile([128, n_half * 1536], F32R)
                for m in range(n_half):
                    rows = min(128, tb - 128 * m)
                    for nn3 in range(3):
                        pg = spp.tile([128, 512], F32, tag="pg")
                        for j in range(4):
                            wtile = stp.tile([128, 512], F32R, tag="wihe")
                            nc.gpsimd.dma_start(wtile[:, :], wihe_dram[128 * j : 128 * (j + 1), 512 * nn3 : 512 * (nn3 + 1)])
                            nc.tensor.matmul(pg[:rows, :], eT[:, tb * j + 128 * m : tb * j + 128 * m + rows], wtile[:, :], start=(j == 0), stop=False)
                        nc.tensor.matmul(pg[:rows, :], ones[:1, :rows], gxeb[:1, 512 * nn3 : 512 * (nn3 + 1)], start=False, stop=True)
                        nc.vector.tensor_copy(gxe[:rows, 1536 * m + 512 * nn3 : 1536 * m + 512 * (nn3 + 1)], pg[:rows, :])
                # epT = Wa @ encT -> [128, (m=r-chunk, bs)]
                epT = sp.tile([128, 4 * 256], F32R)
                for m in range(4):
                    pe_ = spp.tile([128, 256], F32, tag="pe")
                    for j in range(4):
                        nc.tensor.matmul(pe_[:, :], wat[:, 512 * j + 128 * m : 512 * j + 128 * (m + 1)], enct[:, 256 * j : 256 * (j + 1)], start=(j == 0), stop=(j == 3))
                    nc.vector.tensor_copy(epT[:, 256 * m : 256 * (m + 1)], pe_[:, :])
                # enc_wc = enc_r @ W_c^T -> [128, (m, d)]
                enc_wc = sp.tile([128, 2 * 512], F32R)
                for m in range(2):
                    pw = spp.tile([128, 512], F32, tag="pw")
                    for j in range(4):
                        nc.tensor.matmul(pw[:, :], enct[:, 256 * j + 128 * m : 256 * j + 128 * (m + 1)], wct[:, 512 * j : 512 * (j + 1)], start=(j == 0), stop=(j == 3))
                    nc.vector.tensor_copy(enc_wc[:, 512 * m : 512 * (m + 1)], pw[:, :])

            # ---- recurrence ---------------------------------------------
            h_prev = h0sb
            hT_prev = h0t
            with tc.tile_pool(name="st", bufs=2) as stp2, \
                 tc.tile_pool(name="ps_rz", bufs=1, space="PSUM") as prz, \
                 tc.tile_pool(name="ps_n", bufs=1, space="PSUM") as pn, \
                 tc.tile_pool(name="ps_sc", bufs=1, space="PSUM") as psc, \
                 tc.tile_pool(name="ps_bd", bufs=1, space="PSUM") as pbd, \
                 tc.tile_pool(name="ps_o", bufs=1, space="PSUM") as po, \
                 tc.tile_pool(name="ps_t", bufs=1, space="PSUM") as pt_:
                tsb = t_steps * BL
                for t in range(t_steps):
                    tm, tdiv = t % 32, t // 32
                    # 1) gates
                    ps_rz = prz.tile([BL, 1024], F32)
                    ps_n = pn.tile([BL, 1024], F32)
                    for half in range(2):
                        dst = ps_rz[:, 512 * half : 512 * (half + 1)]
                        first = True
                        if t > 0:
                            for j in range(4):
                                nc.tensor.matmul(dst, oT_all[:, tsb * j + 4 * (t - 1) : tsb * j + 4 * t], wiho[:, 1536 * j + 512 * half : 1536 * j + 512 * (half + 1)], start=first, stop=False)
                                first = False
                        for j in range(4):
                            nc.tensor.matmul(dst, hT_prev[:, 4 * j : 4 * (j + 1)], whh[:, 1536 * j + 512 * half : 1536 * j + 512 * (half + 1)], start=first, stop=False)
                            first = False
                        nc.tensor.matmul(dst, i128r[:, 4 * tm : 4 * tm + 4], gxe[:, 1536 * tdiv + 512 * half : 1536 * tdiv + 512 * half + 512], start=False, stop=True)
                    # gxn (o-part + gxe), ghn (h-part + bias)
                    dst = ps_n[:, 0:512]
                    first = True
                    if t > 0:
                        for j in range(4):
                            nc.tensor.matmul(dst, oT_all[:, tsb * j + 4 * (t - 1) : tsb * j + 4 * t], wiho[:, 1536 * j + 1024 : 1536 * j + 1536], start=first, stop=False)
                            first = False
                    nc.tensor.matmul(dst, i128r[:, 4 * tm : 4 * tm + 4], gxe[:, 1536 * tdiv + 1024 : 1536 * tdiv + 1536], start=first, stop=True)
                    dst = ps_n[:, 512:1024]
                    for j in range(4):
                        nc.tensor.matmul(dst, hT_prev[:, 4 * j : 4 * (j + 1)], whh[:, 1536 * j + 1024 : 1536 * j + 1536], start=(j == 0), stop=False)
                    nc.tensor.matmul(dst, ones[:1, :BL], ghnb[:1, :], start=False, stop=True)
                    # 2) sigmoid r|z
                    rz = stp2.tile([BL, 1024], F32, tag="rz")
                    nc.scalar.activation(rz[:, :], ps_rz[:, :], mybir.ActivationFunctionType.Sigmoid)
                    # 3..5) n = tanh(gxn + r*ghn)
                    ntmp = stp2.tile([BL, 512], F32, tag="ntmp")
                    nc.vector.tensor_tensor(ntmp[:, :], rz[:, 0:512], ps_n[:, 512:1024], op=mybir.AluOpType.mult)
                    npre = stp2.tile([BL, 512], F32, tag="npre")
                    nc.vector.tensor_tensor(npre[:, :], ps_n[:, 0:512], ntmp[:, :], op=mybir.AluOpType.add)
                    nsb = stp2.tile([BL, 512], F32, tag="nsb")
                    nc.scalar.activation(nsb[:, :], npre[:, :], mybir.ActivationFunctionType.Tanh)
                    # 6..8) h' = z*h + n - z*n
                    t1 = stp2.tile([BL, 512], F32, tag="t1")
                    nc.vector.tensor_tensor(t1[:, :], rz[:, 512:1024], h_prev[:, :], op=mybir.AluOpType.mult)
                    t4 = stp2.tile([BL, 512], F32, tag="t4")
                    nc.vector.scalar_tensor_tensor(t4[:, :], rz[:, 512:1024], -1.0, nsb[:, :], op0=mybir.AluOpType.mult, op1=mybir.AluOpType.mult)
                    t5 = stp2.tile([BL, 512], F32, tag="t5")
                    nc.vector.tensor_tensor(t5[:, :], t1[:, :], nsb[:, :], op=mybir.AluOpType.add)
                    hnew = stp2.tile([BL, 512], F32, tag="hnew")
                    nc.vector.tensor_tensor(hnew[:, :], t5[:, :], t4[:, :], op=mybir.AluOpType.add)
                    # 9) hT
                    ps_t = pt_.tile([128, 32], F32)
                    for j in range(4):
                        nc.tensor.transpose(ps_t[:, 4 * j : 4 * (j + 1)], hnew[:, 128 * j : 128 * (j + 1)], ident[:BL, :BL])
                    hT = stp2.tile([128, 16], F32R, tag="hT")
                    nc.vector.tensor_copy(hT[:, :], ps_t[:, 0:16])
                    # 10) scores
                    ps_sc = psc.tile([BL, 256], F32)
                    for j in range(4):
                        nc.tensor.matmul(ps_sc[:, :], hT[:, 4 * j : 4 * (j + 1)], epT[:, 256 * j : 256 * (j + 1)], start=(j == 0), stop=(j == 3))
                    # 11) exp (no max-sub; scores bounded)
                    esc = stp2.tile([BL, 256], F32, tag="esc")
                    nc.scalar.activation(esc[:, :], ps_sc[:, :], mybir.ActivationFunctionType.Exp)
                    # 12) diag mask (also applies src_mask)
                    em = stp2.tile([BL, 256], F32, tag="em")
                    nc.vector.tensor_tensor(em[:, :], esc[:, :], dmask[:, :], op=mybir.AluOpType.mult)
                    # 13) denom + recip
                    den = stp2.tile([BL, 1], F32, tag="den")
                    nc.vector.tensor_reduce(den[:, :], em[:, :], axis=mybir.AxisListType.X, op=mybir.AluOpType.add)
                    rden = stp2.tile([BL, 1], F32, tag="rden")
                    nc.vector.reciprocal(rden[:, :], den[:, :])
                    # 14) attn
                    attn = stp2.tile([BL, 256], F32, tag="attn")
                    nc.vector.tensor_scalar_mul(attn[:, :], em[:, :], rden[:, :1])
                    # 15) block-diag via transposes (zeros come from diagmask)
                    ps_bd = pbd.tile([64, 16], F32)
                    for b in range(BL):
                        nc.tensor.transpose(ps_bd[0:64, 4 * b : 4 * (b + 1)], attn[:, 64 * b : 64 * (b + 1)], ident[:BL, :BL])
                    bd = stp2.tile([128, 8], F32R, tag="bd")
                    for b in range(BL):
                        nc.vector.tensor_copy(bd[64 * (b % 2) : 64 * (b % 2) + 64, 4 * (b // 2) : 4 * (b // 2) + 4], ps_bd[0:64, 4 * b : 4 * (b + 1)])
                    # 17) o-proj
                    ps_o = po.tile([BL, 512], F32)
                    for j in range(4):
                        nc.tensor.matmul(ps_o[:, :], hT[:, 4 * j : 4 * (j + 1)], wht[:, 512 * j : 512 * (j + 1)], start=(j == 0), stop=False)
                    for i in range(2):
                        nc.tensor.matmul(ps_o[:, :], bd[:, 4 * i : 4 * (i + 1)], enc_wc[:, 512 * i : 512 * (i + 1)], start=False, stop=False)
                    nc.tensor.matmul(ps_o[:, :], ones[:1, :BL], obias[:1, :], start=False, stop=True)
                    # 18) tanh -> o
                    osb = stp2.tile([BL, 512], F32, tag="osb")
                    nc.scalar.activation(osb[:, :], ps_o[:, :], mybir.ActivationFunctionType.Tanh)
                    # 19) oT
                    for j in range(4):
                        nc.tensor.transpose(ps_t[:, 16 + 4 * j : 16 + 4 * (j + 1)], osb[:, 128 * j : 128 * (j + 1)], ident[:BL, :BL])
                    oT_v = oT_all[:, :].rearrange("p (j t b) -> p j t b", j=4, b=BL)
                    nc.vector.tensor_copy(oT_v[:, :, t, :], ps_t[:, 16:32].rearrange("p (j b) -> p j b", j=4))
                    h_prev = hnew
                    hT_prev = hT

            # ---- logits: out[tb, v_out] = o @ embT + b_logits -----------
            tsb = t_steps * BL
            with tc.tile_pool(name="lw", bufs=6) as lw, \
                 tc.tile_pool(name="lo", bufs=3) as lo, \
                 tc.tile_pool(name="ps_l", bufs=4, space="PSUM") as pl:
                for n in range(NT):
                    ncols = min(512, v_out - 512 * n)
                    wtiles = []
                    for j in range(4):
                        wt = lw.tile([128, 512], F32R, tag="lw")
                        nc.gpsimd.dma_start(wt[:, :ncols], embT_dram[128 * j : 128 * (j + 1), 512 * n : 512 * n + ncols])
                        wtiles.append(wt)
                    blog = lw.tile([1, 512], F32R, tag="blog")
                    nc.gpsimd.dma_start(blog[:1, :ncols], blog_dram[:1, 512 * n : 512 * n + ncols])
                    for m in range(MT):
                        rows = min(128, tb - 128 * m)
                        ps = pl.tile([128, 512], F32, tag="pl")
                        for j in range(4):
                            lhs = oT_all[:, tsb * j + 128 * m : tsb * j + 128 * m + rows]
                            nc.tensor.matmul(ps[:rows, :ncols], lhs, wtiles[j][:, :ncols], start=(j == 0), stop=False)
                        nc.tensor.matmul(ps[:rows, :ncols], ones[:1, :rows], blog[:1, :ncols], start=False, stop=True)
                        lsb = lo.tile([128, 512], F32, tag="lsb")
                        if (n * MT + m) % 2 == 0:
                            nc.vector.tensor_copy(lsb[:rows, :ncols], ps[:rows, :ncols])
                        else:
                            nc.scalar.copy(lsb[:rows, :ncols], ps[:rows, :ncols])
                        nc.sync.dma_start(out_dram[128 * m : 128 * m + rows, 512 * n : 512 * n + ncols], lsb[:rows, :ncols])
    nc.compile()
    return nc


def _host_prep(inputs, t_steps, v_out):
    f32 = np.float32
    trg = np.asarray(inputs["trg"])
    enc = np.asarray(inputs["enc_outputs"], f32)
    h0_all = np.asarray(inputs["init_hidden"], f32)[0]
    mask = np.asarray(inputs["src_mask"], f32)
    emb = np.ascontiguousarray(np.asarray(inputs["emb_W"], f32))
    W_ih = np.asarray(inputs["W_ih"], f32)
    W_hh = np.asarray(inputs["W_hh"], f32)
    b_ih = np.asarray(inputs["b_ih"], f32)
    b_hh = np.asarray(inputs["b_hh"], f32)
    W_a = np.asarray(inputs["W_a"], f32)
    W_h = np.asarray(inputs["W_h"], f32)
    b_h = np.asarray(inputs["b_h"], f32)
    W_c = np.asarray(inputs["W_c"], f32)
    b_c = np.asarray(inputs["b_c"], f32)
    b_logits = np.asarray(inputs["b_logits"], f32)

    embT = np.ascontiguousarray(emb.T[:, :v_out])
    WiheT = np.ascontiguousarray(W_ih[:, :D].T)
    WihoT = np.ascontiguousarray(W_ih[:, D:].T)
    WhhT = np.ascontiguousarray(W_hh.T)
    WhT = np.ascontiguousarray(W_h.T)
    WcT = np.ascontiguousarray(W_c.T)
    WaT = np.ascontiguousarray(W_a.T)
    gxe_bias = (b_ih + np.concatenate([b_hh[: 2 * R], np.zeros(R, f32)])).reshape(1, -1)
    ghn_bias = b_hh[2 * R :].reshape(1, -1)
    o_bias = (b_h + b_c).reshape(1, -1)
    blog = b_logits[:v_out].reshape(1, -1)
    I4 = np.eye(BL, dtype=f32)
    I128 = np.eye(128, dtype=f32)

    in_maps = []
    for c in range(NC_):
        bs = slice(BL * c, BL * (c + 1))
        trg_l = trg[bs, :t_steps]
        idx = np.ascontiguousarray(trg_l.T.reshape(-1, 1).astype(np.int32))
        enc_l = enc[bs]
        enc_r = np.ascontiguousarray(enc_l.reshape(BL * S, E))
        encT = np.ascontiguousarray(enc_r.T)
        dmask = np.zeros((BL, BL * S), f32)
        for b in range(BL):
            dmask[b, S * b : S * (b + 1)] = mask[BL * c + b]
        h0 = np.ascontiguousarray(h0_all[bs])
        h0T = np.zeros((128, 4 * BL), f32)
        for j in range(4):
            for b in range(BL):
                h0T[:, 4 * j + b] = h0[b, 128 * j : 128 * (j + 1)]
        in_maps.append({
            "emb_W": emb, "embT": embT, "idx": idx, "WiheT": WiheT,
            "WihoT": WihoT, "WhhT": WhhT, "WhT": WhT, "WcT": WcT, "WaT": WaT,
            "enc_r": enc_r, "encT": encT, "gxe_bias": gxe_bias,
            "ghn_bias": ghn_bias, "o_bias": o_bias, "b_logits": blog,
            "diagmask": dmask, "h0": h0, "h0T": h0T, "I4": I4, "I128": I128,
            "ones128": np.ones((1, 128), f32),
        })
    return in_maps


def kernel(t_steps=T, v_out=V, trace=False, **inputs):
    key = (t_steps, v_out)
    if key not in _BUILT:
        _BUILT[key] = build_kernel(t_steps, v_out)
    nc = _BUILT[key]
    in_maps = _host_prep(inputs, t_steps, v_out)
    res = run_bass_kernel_spmd(nc, in_maps, core_ids=list(range(NC_)), trace=trace)
    out = np.zeros((B, t_steps, v_out), np.float32)
    for c in range(NC_):
        r = res.results[c]["out"].reshape(t_steps, BL, v_out)
        out[BL * c : BL * (c + 1)] = r.transpose(1, 0, 2)
    kernel.last_exec_ns = res.exec_time_ns
    return out
